# revision 44
# baseline (speedup 1.0000x reference)
"""MoE (ExpertPool) expert-parallel kernel for Trainium2, 8 NeuronCores.

Strategy (per sharding hint): expert-parallel with chunk-granular load
balancing. Host computes the (tiny) router: logits = x@Wr+br, top-2
selection, softmax combine weights. The 32768 routed (token, expert) pairs
are packed into fixed-size token chunks (each chunk single-expert, zero
padding gated off), and chunks are distributed across the 8 cores so every
core processes the same C tokens (C ~ ceil(32768/8) instead of the max
expert load). Each chunk carries its own expert's weight panels (weights
are re-streamed from HBM per chunk anyway, so per-chunk weight copies add
no DMA traffic). Host scatter-adds the per-chunk outputs back ("combine").

Device kernel layout: everything feature-major (features on SBUF
partitions, tokens on the free dim), fp16 datapath with fp32 PSUM/bias/
gate/output. fp16 (vs float32r) halves the per-matmul weight-load time on
the PE — with f32r the 128x128 LDWEIGHTS (~190ns) was longer than a
384-column stream (~160ns), exposing ~90us of load time per core — and
halves all weight/activation DMA traffic. End-to-end error vs the fp64
oracle is ~5e-4 (same as f32r: both are dominated by 3-layer rounding).

Per chunk, W1/W2/W3 stream through SBUF as host-pretiled, fully-contiguous
128-column panels; matmul column groups are 256..512 wide (one single-bank
PSUM accumulator each; matmuls may not cross PSUM banks), wide enough to
hide the fp16 LDWEIGHTS (~97ns) under the moving-data stream. Chunks run
smallest-first: the first chunk's x/W1 DMAs gate PE start, and each
chunk's L2+L3 phase is the W1-prefetch window for the next chunk. Rings:
W1 panels ride the gpsimd HWDGE ring (chunk 0 alternates gpsimd/sync to
feed L1 at startup; later chunks keep gpsimd exclusively so next-chunk W1
prefetch never queues behind W2/W3), W2 rides sync — except chunk 0's
first panels, which ride gpsimd so the L1->L2 transition isn't stalled
behind chunk 0's odd W1 panels — and W3 + x + gate + y-stores ride scalar.
GELU+bias fuse into ScalarE activations reading PSUM; gating is a DVE
multiply against a partition-broadcast gate row; next-chunk x tiles
prefetch during L2. Output is written feature-major and transposed on the
host during combine. Measured ~0.90 ms end-to-end on 8 cores (~95% MFU on
the traced core vs the 78.6 TF/s fp16 PE roofline); fp8 was evaluated and
rejected: DoubleRow fp8 measures 2x (not 4x), and naive fp8 quantization
of any single layer costs ~4e-2 relative error, over the 2e-2 gate.
"""

import numpy as np

# Problem dims (hardcoded per spec: nn_ExpertPool_8366596292698)
B, S, D, E, I = 8, 2048, 768, 8, 3072
H = I // 2
T = B * S
P = 128
KD, KI, KH = D // P, I // P, H // P  # 6, 24, 12
N_CORES = 8

_PROGRAM_CACHE: dict = {}
_PLAN_CACHE: dict = {}
LAST_RESULTS = None  # BassKernelResults of the most recent run (for test harness)

# chunk sizes the device program supports: single-bank PSUM groups of
# 256..512 columns (one group for <=512, an even split for larger).
# 16-token granularity keeps total padding (8C - routed pairs) small.
_CHUNK_SIZES = tuple(range(1024, 255, -16))
_GRAIN = 16


def _col_groups(nc_tokens):
    if nc_tokens <= 512:
        return [(0, nc_tokens)]
    h = nc_tokens // 2
    return [(0, h), (h, nc_tokens - h)]


def _min_group(sizes):
    return min(min(cn for _, cn in _col_groups(s)) for s in sizes)


def _order_chunks(sizes):
    """Ascending: smallest chunk first (its x DMA gates PE start), and each
    chunk's L2+L3 phase — the W1-prefetch window for the next chunk — only
    grows as the chunks get bigger."""
    return sorted(sizes)


def _enum_lists(C, n_chunks, cap=100000):
    """Multisets of _CHUNK_SIZES of exactly n_chunks summing to C
    (non-increasing order)."""
    out = []
    smin, smax = _CHUNK_SIZES[-1], _CHUNK_SIZES[0]

    def rec(rem, start, cur):
        if len(out) >= cap:
            return
        k = n_chunks - len(cur)
        if k == 0:
            if rem == 0:
                out.append(tuple(cur))
            return
        for i in range(start, len(_CHUNK_SIZES)):
            s = _CHUNK_SIZES[i]
            if s * k < rem:
                break  # sizes descend: the rest are too small to reach C
            if s > rem or rem - s < smin * (k - 1):
                continue
            cur.append(s)
            rec(rem - s, i, cur)
            cur.pop()

    rec(C, 0, [])
    return out


def _try_assign(counts, sizes):
    """Assign experts to chunk slots (8 copies of `sizes`). Returns
    {expert: multiset-of-sizes} or None."""
    from collections import Counter

    mult = Counter()
    for s in sizes:
        mult[s] += N_CORES
    order = sorted(range(len(counts)), key=lambda e: -counts[e])
    assign = {}
    nodes = [0]

    def combos(need, avail):
        """Size-multisets covering `need` with bounded overshoot."""
        szs = sorted(avail, reverse=True)
        seen = []

        def rec(rem, start, cur):
            if len(seen) > 60:
                return
            if rem <= 0:
                seen.append(tuple(cur))
                return
            for i in range(start, len(szs)):
                s = szs[i]
                if cur.count(s) >= avail[s]:
                    continue
                if rem - s <= 0 and s - rem > 448:
                    continue  # excessive overshoot
                cur.append(s)
                rec(rem - s, i, cur)
                cur.pop()

        rec(need, 0, [])
        # smallest overshoot first
        return sorted(seen, key=lambda c: sum(c) - need)

    def dfs(idx):
        nodes[0] += 1
        if nodes[0] > 20000:
            return False
        if idx == len(order):
            return True
        e = order[idx]
        needed_rest = sum(counts[order[i]] for i in range(idx + 1, len(order)))
        for combo in combos(counts[e], mult):
            cc = Counter(combo)
            if any(mult[s] < n for s, n in cc.items()):
                continue
            cap_left = sum(mult[s] * s for s in mult) - sum(combo)
            if cap_left < needed_rest:
                continue  # remaining slots can't cover remaining experts
            for s, n in cc.items():
                mult[s] -= n
            assign[e] = combo
            if dfs(idx + 1):
                return True
            del assign[e]
            for s, n in cc.items():
                mult[s] += n
        return False

    return assign if dfs(0) else None


def _plan_schedule(counts):
    """Pick (chunks, plan): per-core chunk-size list and per-slot expert.

    plan[core][chunk_idx] = expert id or -1 (padding slot)."""
    key = tuple(int(c) for c in counts)
    if key in _PLAN_CACHE:
        return _PLAN_CACHE[key]
    import time as _time

    total = int(sum(counts))
    # per-expert coverage must overshoot to a _GRAIN multiple, so total
    # slack 8C - total must at least cover the sum of those remainders
    min_slack = sum((_GRAIN - c % _GRAIN) % _GRAIN for c in counts)
    lo = -(-total // (N_CORES * _GRAIN)) * _GRAIN
    best = None
    deadline = _time.monotonic() + 10.0  # give up -> per-expert fallback
    for C in range(lo, lo + 512 + 1, _GRAIN):
        if best or _time.monotonic() > deadline:
            break
        if C * N_CORES - total < min_slack:
            continue
        # fewest chunks first (less weight restreaming), then wide groups
        for n in range(-(-C // _CHUNK_SIZES[0]), 7):
            lists = _enum_lists(C, n)
            lists.sort(key=lambda L: (-_min_group(L), -min(L)))
            for L in lists[:300]:
                if _min_group(L) < 256:
                    continue
                if _time.monotonic() > deadline:
                    break
                a = _try_assign(list(counts), L)
                if a is not None:
                    best = (list(L), a)
                    break
            if best or _time.monotonic() > deadline:
                break
        if best:
            break
    if best is None:
        # fallback: one expert per core
        C = max(256, int(-(-max(counts) // P) * P))
        k, r = divmod(C, 768)
        if r == 0:
            L = [768] * k
        elif r >= 256:
            L = [r] + [768] * k
        else:
            L = [448, 448] + [768] * (k - 1)
        plan_sizes = _order_chunks(L)
        plan = [[e] * len(plan_sizes) for e in range(N_CORES)]
        _PLAN_CACHE[key] = (plan_sizes, plan, False)
        return _PLAN_CACHE[key]

    sizes, assign = best
    sizes_ord = _order_chunks(sizes)
    # physical slots: per core, the ordered chunk list; fill greedily
    from collections import Counter, defaultdict

    free = defaultdict(list)  # size -> [(core, idx)]
    for core in range(N_CORES):
        for j, s in enumerate(sizes_ord):
            free[s].append((core, j))
    plan = [[-1] * len(sizes_ord) for _ in range(N_CORES)]
    for e in sorted(assign):
        for s in assign[e]:
            core, j = free[s].pop()
            plan[core][j] = e
    _PLAN_CACHE[key] = (sizes_ord, plan, True)
    return _PLAN_CACHE[key]


def _build_program(chunks, has_b1, has_b2, has_b3):
    from contextlib import ExitStack

    import concourse.bacc as bacc
    import concourse.bass as bass
    import concourse.mybir as mybir
    import concourse.tile as tile

    f32 = mybir.dt.float32
    f16 = mybir.dt.float16
    GELU = mybir.ActivationFunctionType.Gelu

    C = sum(chunks)
    NCH = len(chunks)

    nc = bacc.Bacc(
        "TRN2",
        target_bir_lowering=False,
        debug=False,
        enable_asserts=False,
        num_devices=N_CORES,
    )

    # host-pretiled layouts: every DMA below reads/writes one fully
    # contiguous block. Weights are per-chunk (each chunk may belong to a
    # different expert).
    xT = nc.dram_tensor("xTt", [D * C], f16, kind="ExternalInput").ap()
    w1 = nc.dram_tensor("w1t", [NCH, KI, P, KD * P], f16, kind="ExternalInput").ap()
    w2 = nc.dram_tensor("w2t", [NCH, KH, P, KI * P], f16, kind="ExternalInput").ap()
    w3 = nc.dram_tensor("w3t", [NCH, KD, P, KH * P], f16, kind="ExternalInput").ap()
    gate = nc.dram_tensor("gate", [C], f32, kind="ExternalInput").ap()
    b1 = b2 = b3 = None
    if has_b1:
        b1 = nc.dram_tensor("b1t", [NCH, P, KI], f32, kind="ExternalInput").ap()
    if has_b2:
        b2 = nc.dram_tensor("b2t", [NCH, P, KH], f32, kind="ExternalInput").ap()
    if has_b3:
        b3 = nc.dram_tensor("b3t", [NCH, P, KD], f32, kind="ExternalInput").ap()
    yT = nc.dram_tensor("yTt", [D * C], f32, kind="ExternalOutput").ap()

    with tile.TileContext(nc) as tc, ExitStack() as ctx:
        bpool = ctx.enter_context(tc.tile_pool(name="bias", bufs=2))
        xpool = ctx.enter_context(tc.tile_pool(name="x", bufs=1))
        h1pool = ctx.enter_context(tc.tile_pool(name="h1", bufs=1))
        h2pool = ctx.enter_context(tc.tile_pool(name="h2", bufs=1))
        w1pool = ctx.enter_context(tc.tile_pool(name="w1p", bufs=12))
        w2pool = ctx.enter_context(tc.tile_pool(name="w2p", bufs=4))
        w3pool = ctx.enter_context(tc.tile_pool(name="w3p", bufs=3))
        ypool = ctx.enter_context(tc.tile_pool(name="y", bufs=2))
        gpool = ctx.enter_context(tc.tile_pool(name="g", bufs=2))
        pspool = ctx.enter_context(
            tc.tile_pool(name="ps", bufs=8, space=bass.MemorySpace.PSUM)
        )

        # chunk start offsets
        bases = []
        b_ = 0
        for s in chunks:
            bases.append(b_)
            b_ += s

        def load_chunk_inputs(ci):
            """DMA this chunk's token activations + gate row + biases on
            the scalar ring (x tiles land in consumption order, ahead of
            the PE's k-loop)."""
            Nc = chunks[ci]
            base = bases[ci]
            x_sb = []
            for k in range(KD):
                xk = xpool.tile([P, Nc], f16, tag=f"x{k}")
                off = (base * D) + k * P * Nc
                eng = (nc.scalar, nc.sync)[k % 2] if ci == 0 else nc.scalar
                eng.dma_start(
                    xk[:], xT[off : off + P * Nc].rearrange("(p f) -> p f", f=Nc)
                )
                x_sb.append(xk)
            g_bc = gpool.tile([P, Nc], f32, tag="gbc")
            nc.scalar.dma_start(
                g_bc[:],
                gate[base : base + Nc].unsqueeze(0).partition_broadcast(P).squeeze(1),
            )
            b_sb = [None, None, None]
            if has_b1:
                b_sb[0] = bpool.tile([P, KI], f32, tag="b1")
                nc.scalar.dma_start(b_sb[0][:], b1[ci])
            if has_b2:
                b_sb[1] = bpool.tile([P, KH], f32, tag="b2")
                nc.scalar.dma_start(b_sb[1][:], b2[ci])
            if has_b3:
                b_sb[2] = bpool.tile([P, KD], f32, tag="b3")
                nc.scalar.dma_start(b_sb[2][:], b3[ci])
            return x_sb, g_bc, b_sb

        pending = load_chunk_inputs(0)
        for ci, Nc in enumerate(chunks):
            base = bases[ci]
            cgs = _col_groups(Nc)
            x_sb, g_bc, (b1_sb, b2_sb, b3_sb) = pending

            # ---- L1: h1 = gelu(x @ W1 + b1), feature-major [I, Nc] ----
            h1_sb = []
            for m in range(KI):
                w1p = w1pool.tile([P, KD * P], f16, tag="w1p")
                # chunk 0 L1 is startup-critical: feed panels on two rings
                eng = (nc.gpsimd, nc.sync)[m % 2] if ci == 0 else nc.gpsimd
                eng.dma_start(w1p[:], w1[ci, m])
                h1m = h1pool.tile([P, Nc], f16, tag=f"h1_{m}")
                for cs, cn in cgs:
                    ps = pspool.tile([P, cn], f32, tag="ps")
                    for k in range(KD):
                        nc.tensor.matmul(
                            ps[:],
                            lhsT=w1p[:, k * P : (k + 1) * P],
                            rhs=x_sb[k][:, cs : cs + cn],
                            start=(k == 0),
                            stop=(k == KD - 1),
                        )
                    nc.scalar.activation(
                        h1m[:, cs : cs + cn],
                        ps[:],
                        GELU,
                        bias=(b1_sb[:, m : m + 1] if has_b1 else 0.0),
                    )
                h1_sb.append(h1m)

            # prefetch next chunk's activations; their x slots free as soon
            # as this chunk's L1 matmuls finish, so the DMA lands during L2
            if ci + 1 < len(chunks):
                pending = load_chunk_inputs(ci + 1)

            # ---- L2: h2 = gelu(h1 @ W2 + b2), feature-major [H, Nc] ----
            h2_sb = []
            for m in range(KH):
                w2p = w2pool.tile([P, KI * P], f16, tag="w2p")
                # chunk 0's first W2 panels ride gpsimd (idle once chunk 0's
                # even W1 panels are in): on sync they'd queue behind the 12
                # odd W1 panels and stall the PE at the L1->L2 transition
                eng = nc.gpsimd if (ci == 0 and m < 3) else nc.sync
                eng.dma_start(w2p[:], w2[ci, m])
                h2m = h2pool.tile([P, Nc], f16, tag=f"h2_{m}")
                for cs, cn in cgs:
                    ps = pspool.tile([P, cn], f32, tag="ps")
                    for k in range(KI):
                        nc.tensor.matmul(
                            ps[:],
                            lhsT=w2p[:, k * P : (k + 1) * P],
                            rhs=h1_sb[k][:, cs : cs + cn],
                            start=(k == 0),
                            stop=(k == KI - 1),
                        )
                    nc.scalar.activation(
                        h2m[:, cs : cs + cn],
                        ps[:],
                        GELU,
                        bias=(b2_sb[:, m : m + 1] if has_b2 else 0.0),
                    )
                h2_sb.append(h2m)

            # ---- L3: y = (h2 @ W3 + b3) * gate, feature-major [D, Nc] ----
            for m in range(KD):
                w3p = w3pool.tile([P, KH * P], f16, tag="w3p")
                nc.scalar.dma_start(w3p[:], w3[ci, m])
                y_sb = ypool.tile([P, Nc], f32, tag="y")
                for cs, cn in cgs:
                    ps = pspool.tile([P, cn], f32, tag="ps")
                    for k in range(KH):
                        nc.tensor.matmul(
                            ps[:],
                            lhsT=w3p[:, k * P : (k + 1) * P],
                            rhs=h2_sb[k][:, cs : cs + cn],
                            start=(k == 0),
                            stop=(k == KH - 1),
                        )
                    if has_b3:
                        nc.vector.tensor_scalar_add(
                            y_sb[:, cs : cs + cn], ps[:], b3_sb[:, m : m + 1]
                        )
                        nc.vector.tensor_mul(
                            y_sb[:, cs : cs + cn],
                            y_sb[:, cs : cs + cn],
                            g_bc[:, cs : cs + cn],
                        )
                    else:
                        nc.vector.tensor_mul(
                            y_sb[:, cs : cs + cn], ps[:], g_bc[:, cs : cs + cn]
                        )
                    # per-group store (group-major DRAM layout keeps each
                    # transfer contiguous): the first group of the final
                    # panel streams out while the second is still in DVE,
                    # shortening the end-of-program drain
                    yoff = (base * D) + m * P * Nc + cs * P
                    nc.scalar.dma_start(
                        yT[yoff : yoff + P * cn].rearrange("(p f) -> p f", f=cn),
                        y_sb[:, cs : cs + cn],
                    )

    nc.compile()
    return nc


def _route(x, Wr, br, top_k):
    """Host router: fp32 logits, stable top-k, softmax weights."""
    xt = np.ascontiguousarray(x.reshape(T, D), dtype=np.float32)
    logits = (xt @ np.asarray(Wr, np.float32)) + np.asarray(br, np.float32)
    k = int(top_k)
    # descending by value, ties -> lower index (matches jax.lax.top_k)
    order = np.argsort(-logits, axis=1, kind="stable")[:, :k]  # [T, k]
    vals = np.take_along_axis(logits, order, axis=1)
    vmax = vals.max(axis=1, keepdims=True)
    ex = np.exp(vals - vmax)
    wts = (ex / ex.sum(axis=1, keepdims=True)).astype(np.float32)
    return xt, order, wts


def _tile_w(w, km):
    """[K, M] -> [km_panels, 128, K] panel-contiguous fp16 layout."""
    K, M = w.shape
    # panel m: element (p, a*128+f) = w[a*128+p, m*128+f]
    v = w.reshape(K // P, P, km, P)  # [a, p, m, f]
    return np.ascontiguousarray(v.transpose(2, 1, 0, 3).astype(np.float16)).reshape(
        km, P, K
    )


def kernel(x, Wr, br, W1, b1, W2, b2, W3, b3, top_k):
    global LAST_RESULTS
    import os

    from concourse import bass_utils

    x = np.asarray(x)
    out_dtype = x.dtype
    xt, sel, wts = _route(x, Wr, br, top_k)

    W1 = np.asarray(W1, np.float32)
    W2 = np.asarray(W2, np.float32)
    W3 = np.asarray(W3, np.float32)
    b1 = np.asarray(b1, np.float32)
    b2 = np.asarray(b2, np.float32)
    b3 = np.asarray(b3, np.float32)

    # token lists per expert
    idx_e = []
    gate_e = []
    for e in range(E):
        rows, cols = np.nonzero(sel == e)
        idx_e.append(rows)
        gate_e.append(wts[rows, cols])
    counts = [len(i) for i in idx_e]

    chunks, plan, _balanced = _plan_schedule(counts)
    C = sum(chunks)
    NCH = len(chunks)

    has_b1 = bool(np.any(b1))
    has_b2 = bool(np.any(b2))
    has_b3 = bool(np.any(b3))

    key = (tuple(chunks), has_b1, has_b2, has_b3)
    if key not in _PROGRAM_CACHE:
        _PROGRAM_CACHE[key] = _build_program(list(chunks), has_b1, has_b2, has_b3)
    nc = _PROGRAM_CACHE[key]

    bases = []
    b_ = 0
    for s in chunks:
        bases.append(b_)
        b_ += s

    # split each expert's token list across its assigned slots, in
    # (core, chunk) scan order for determinism
    slot_rows = [[None] * NCH for _ in range(N_CORES)]  # token-row arrays
    used = [0] * E
    for core in range(N_CORES):
        for j in range(NCH):
            e = plan[core][j]
            if e < 0:
                continue
            n = min(chunks[j], counts[e] - used[e])
            slot_rows[core][j] = idx_e[e][used[e] : used[e] + n]
            used[e] += n
    assert all(used[e] == counts[e] for e in range(E)), (used, counts)

    t1 = [_tile_w(W1[e], KI) for e in range(E)]
    t2 = [_tile_w(W2[e], KH) for e in range(E)]
    t3 = [_tile_w(W3[e], KD) for e in range(E)]
    tb1 = [np.ascontiguousarray(b1[e].reshape(KI, P).T) for e in range(E)]
    tb2 = [np.ascontiguousarray(b2[e].reshape(KH, P).T) for e in range(E)]
    tb3 = [np.ascontiguousarray(b3[e].reshape(KD, P).T) for e in range(E)]

    in_maps = []
    for core in range(N_CORES):
        xflat = np.zeros((D * C,), np.float16)
        g = np.zeros((C,), np.float32)  # filled after the loop below
        es = [max(plan[core][j], 0) for j in range(NCH)]
        for j in range(NCH):
            e = plan[core][j]
            if e < 0:
                continue
            rows = slot_rows[core][j]
            n = len(rows)
            base, Nc = bases[j], chunks[j]
            blk = np.zeros((D, Nc), np.float16)
            blk[:, :n] = xt[rows].T
            xflat[base * D : (base + Nc) * D] = blk.reshape(-1)
        m = {
            "xTt": xflat,
            "w1t": np.stack([t1[e] for e in es]),
            "w2t": np.stack([t2[e] for e in es]),
            "w3t": np.stack([t3[e] for e in es]),
            "gate": g,
        }
        if has_b1:
            m["b1t"] = np.stack([tb1[e] for e in es])
        if has_b2:
            m["b2t"] = np.stack([tb2[e] for e in es])
        if has_b3:
            m["b3t"] = np.stack([tb3[e] for e in es])
        in_maps.append(m)

    # fill gates (gate_e[e] aligned with idx_e[e]; slots consumed in order)
    used = [0] * E
    for core in range(N_CORES):
        for j in range(NCH):
            e = plan[core][j]
            if e < 0:
                continue
            n = len(slot_rows[core][j])
            base = bases[j]
            in_maps[core]["gate"][base : base + n] = gate_e[e][
                used[e] : used[e] + n
            ]
            used[e] += n

    trace_cores = None
    if os.environ.get("BASS_TRACE"):
        trace_cores = [0]

    res = bass_utils.run_bass_kernel_spmd(
        nc,
        in_maps,
        core_ids=list(range(N_CORES)),
        trace_cores=trace_cores,
    )
    LAST_RESULTS = res

    out = np.zeros((T, D), np.float32)
    for e in range(E):  # expert-ascending to match reference summation order
        for core in range(N_CORES):
            for j in range(NCH):
                if plan[core][j] != e:
                    continue
                rows = slot_rows[core][j]
                n = len(rows)
                if n == 0:
                    continue
                base, Nc = bases[j], chunks[j]
                flat = res.results[core]["yTt"]
                # per panel: column groups stored group-major, each [P, cn]
                blk = np.empty((D, Nc), np.float32)
                for m in range(KD):
                    po = base * D + m * P * Nc
                    for cs, cn in _col_groups(Nc):
                        blk[m * P : (m + 1) * P, cs : cs + cn] = flat[
                            po + cs * P : po + (cs + cn) * P
                        ].reshape(P, cn)
                out[rows] += blk[:, :n].T
    return out.reshape(B, S, D).astype(out_dtype, copy=False)
